# revision 31
# baseline (speedup 1.0000x reference)
"""Trainium2 Bass kernel for nn_BertGTHead_37177236914708 (BertGT pooling head).

Full-input contract: kernel(**inputs) takes the complete (unsharded) numpy
inputs and returns the full [B, 1+G] float32 output.

Strategy (data-parallel over batch, 2 examples per NeuronCore, 8 cores):
  - x is shipped to DRAM as bf16 (tolerance is 2e-2; bf16 keeps us ~1e-3),
    halving HBM traffic and doubling VectorE throughput. All streaming is
    plain HWDGE (sync) DMA - no SWDGE emission on the critical path.
  - text pooling streams [128, 4-row] chunks; masked sums accumulate on the
    PE with the bf16 0/1 mask column as the stationary operand (start/stop
    PSUM groups); the masked values for the running max are produced by
    per-partition-scalar multiplies split between ScalarE (3 of 4 slots)
    and VectorE tensor_scalar at 4x (1 slot); running elementwise max on
    VectorE in bf16 (exact); partition-axis max finalized via PE transposes
    + one free-axis reduce.
  - the text-sum [1, H] PSUM row is transposed (6 tiny PE transposes) into
    h-partitioned layout so the cls dot is a single [128, 18] mult+reduce.
  - window pooling: 32-row padded windows gathered by ONE indirect DMA as
    [(ob,ex,g) partitions x 8 whole rows]; mask applied via VectorE
    tensor_scalar (exact 0/1); within-partition sum/max TT trees in bf16;
    cross-block reduction via PE transposes to h-partitioned layout; center
    rows by a second tiny indirect gather, also transposed.
  - final scores: one combined per-partition dot (center|max|avg vs relaid
    weights) reduced on VectorE, then a single ones-matmul sums the 128
    h-partials for all 34 outputs at once; host unpacks the packed row.
  - all small aux inputs (indices, masks, weights) ride ONE packed DMA.

Everything index/mask-shaped is precomputed on the host; all O(B*S*H) math
runs on the NeuronCores.
"""

import numpy as np
from contextlib import ExitStack

# ---- problem constants (hardcoded; harness runs kernel.py standalone) ----
B, S, H, G = 16, 4096, 768, 16
WIN = 15
WLEN = 2 * WIN + 1           # 31
NCORES = 8
EX = B // NCORES             # 2 examples per core
P = 128
GRP = 4                      # token rows per partition per stream chunk
NCH = S // (P * GRP)         # 8 stream chunks per example
OB = 4                       # 8-row blocks per (32-row padded) window
OB_R = 8                     # rows per block
NE = EX * G                  # 32 (ex, gap) pairs per core
NOUT = 1 + G                 # 17 scores per example
NMC = EX * NCH * GRP         # 64 stream mask columns

_BUILT = None
DEBUG = False


def _aux_cols():
    """Column offsets of the packed [P, K] i32 aux tensor (widths in i32)."""
    c = {}
    o = 0
    for name, w in [("winidx", 1), ("ctridx", 1), ("invcnt", NE),
                    ("gwt", 18), ("pooledr", EX * 6), ("cwc", EX * 18),
                    ("tmaskf", NMC), ("tmaskb", NMC // 2),
                    ("wmaskf", OB_R)]:
        c[name] = (o, o + w)
        o += w
    return c, o


def _build():
    """Build + compile the per-core Bass program (cached)."""
    global _BUILT
    if _BUILT is not None:
        return _BUILT

    import concourse.bacc as bacc
    import concourse.bass as bass
    import concourse.tile as tile
    from concourse import mybir
    from concourse.masks import make_identity

    f32 = mybir.dt.float32
    bf16 = mybir.dt.bfloat16
    i32 = mybir.dt.int32
    AF = mybir.ActivationFunctionType
    OP = mybir.AluOpType
    AX = mybir.AxisListType
    IOA = bass.IndirectOffsetOnAxis

    COLS, NAUX = _aux_cols()

    nc = bacc.Bacc("TRN2", target_bir_lowering=False, debug=False,
                   num_devices=NCORES)

    x_d = nc.dram_tensor("x", [EX * S, H], bf16, kind="ExternalInput").ap()
    aux_d = nc.dram_tensor("aux", [P, NAUX], i32, kind="ExternalInput").ap()
    out_d = nc.dram_tensor("out", [NE + EX], f32, kind="ExternalOutput").ap()

    with tile.TileContext(nc) as tc, ExitStack() as ctx:
        singles = ctx.enter_context(tc.tile_pool(name="singles", bufs=1))
        xpool = ctx.enter_context(tc.tile_pool(name="xin", bufs=6))
        xmpool = ctx.enter_context(tc.tile_pool(name="xm", bufs=3))
        accpool = ctx.enter_context(tc.tile_pool(name="acc", bufs=2))
        winpool = ctx.enter_context(tc.tile_pool(name="win", bufs=1))
        smalls = ctx.enter_context(tc.tile_pool(name="smalls", bufs=4))
        foldp = ctx.enter_context(tc.tile_pool(name="fold", bufs=4))
        pacc = ctx.enter_context(tc.tile_pool(name="pacc", bufs=2, space="PSUM"))
        pacc2 = ctx.enter_context(tc.tile_pool(name="pacc2", bufs=1, space="PSUM"))
        pbig = ctx.enter_context(tc.tile_pool(name="pbig", bufs=2, space="PSUM"))
        pbigc = ctx.enter_context(tc.tile_pool(name="pbigc", bufs=1, space="PSUM"))

        # ---- ONE packed aux dma (first: it gates the window gathers) ----
        aux_sb = singles.tile([P, NAUX], i32)
        nc.sync.dma_start(out=aux_sb[:], in_=aux_d)

        def aux(name, dtype=None, rows=None):
            lo, hi = COLS[name]
            ap = aux_sb[0:rows, lo:hi] if rows else aux_sb[:, lo:hi]
            return ap.bitcast(dtype) if dtype else ap

        winidx_sb = aux("winidx")
        ctridx_sb = aux("ctridx", rows=NE)
        invcnt_sb = aux("invcnt", f32)
        gwt_sb = aux("gwt", f32)
        pooledr_sb = aux("pooledr", f32)
        cwc_sb = aux("cwc", f32)
        tmaskf_sb = aux("tmaskf", f32)       # [P, NMC] f32 (ACT scale)
        tmaskb_sb = aux("tmaskb", bf16)      # [P, NMC] bf16 (PE lhsT / DVE TS)
        wmaskf_sb = aux("wmaskf", f32)       # [P, OB_R] f32

        # rhs of the final ones-matmul: cols 0..NE-1 = per-(ex,g) gap-score
        # partials (per h'-partition), col NE+ex = cls partials
        rhs34 = smalls.tile([P, NE + EX], f32)

        xrow = bass.AP(x_d.tensor, 0, [[H, EX * S], [1, H]])
        x3 = bass.AP(x_d.tensor, 0, [[GRP * H, EX * S // GRP], [1, GRP * H]])

        # ---- window gathers FIRST on the Pool queue, so wt lands while
        # VectorE is still idle and the window trees fill the warm-up gap
        wt = winpool.tile([P, OB_R * H], bf16)
        nc.gpsimd.indirect_dma_start(
            out=wt[:], out_offset=None, in_=xrow,
            in_offset=IOA(ap=winidx_sb, axis=0))
        ct = winpool.tile([NE, H], bf16)
        nc.gpsimd.indirect_dma_start(
            out=ct[:], out_offset=None, in_=xrow,
            in_offset=IOA(ap=ctridx_sb, axis=0))

        ident_bf = singles.tile([P, P], bf16)
        make_identity(nc, ident_bf[:])
        ones_f = singles.tile([P, 1], f32)
        nc.vector.memset(ones_f[:], 1.0)

        with nc.allow_low_precision(reason="bf16 pooling; tol 2e-2"):
            # window mask-mul + trees on GpSimd: it is idle in the stream
            # phase while VectorE is the bottleneck engine
            for o in range(OB_R):
                nc.vector.tensor_scalar_mul(out=wt[:, o * H:(o + 1) * H],
                                            in0=wt[:, o * H:(o + 1) * H],
                                            scalar1=wmaskf_sb[:, o:o + 1])

            # ---- streaming text-pooling phase ----
            for ex in range(EX):
                acc = accpool.tile([P, GRP * H], bf16)
                ps = pacc.tile([1, H], f32, tag="ps")
                for T in range(NCH):
                    xt = xpool.tile([P, GRP * H], bf16, tag="xt")
                    row0 = (ex * S + T * P * GRP) // GRP
                    nc.sync.dma_start(out=xt[:], in_=x3[row0:row0 + P, :])
                    xm = acc if T == 0 else xmpool.tile([P, GRP * H], bf16,
                                                        tag="xm")
                    first = T == 0
                    last = T == NCH - 1
                    for j in range(GRP):
                        c = (ex * NCH + T) * GRP + j
                        mb = tmaskb_sb[:, c:c + 1]
                        nc.tensor.matmul(out=ps[0:1, 0:512], lhsT=mb,
                                         rhs=xt[:, j * H:j * H + 512],
                                         start=first and j == 0,
                                         stop=last and j == GRP - 1)
                        nc.tensor.matmul(out=ps[0:1, 512:H], lhsT=mb,
                                         rhs=xt[:, j * H + 512:(j + 1) * H],
                                         start=first and j == 0,
                                         stop=last and j == GRP - 1)
                        dst = xm[:, j * H:(j + 1) * H]
                        src = xt[:, j * H:(j + 1) * H]
                        if j == GRP - 1:
                            nc.vector.tensor_scalar_mul(
                                out=dst, in0=src,
                                scalar1=tmaskf_sb[:, c:c + 1])
                        else:
                            nc.scalar.activation(out=dst, in_=src,
                                                 func=AF.Copy,
                                                 scale=tmaskf_sb[:, c:c + 1])
                    if T > 0:
                        nc.vector.tensor_tensor(out=acc[:], in0=acc[:],
                                                in1=xm[:], op=OP.max)

                # fold the GRP slots: maxf[p, h] = max_j acc[p, j*H + h]
                nc.vector.tensor_tensor(out=acc[:, 0:2 * H],
                                        in0=acc[:, 0:2 * H],
                                        in1=acc[:, 2 * H:4 * H], op=OP.max)
                maxf = foldp.tile([P, H], bf16)
                nc.vector.tensor_tensor(out=maxf[:], in0=acc[:, 0:H],
                                        in1=acc[:, H:2 * H], op=OP.max)

                # transpose to h-partition layout; reduce the 128 token rows
                pt = pbig.tile([P, H], bf16, tag="ptw")
                for c in range(6):
                    nc.tensor.transpose(out=pt[:, c * P:(c + 1) * P],
                                        in_=maxf[:, c * P:(c + 1) * P],
                                        identity=ident_bf[:])
                feat = foldp.tile([P, 18], f32)
                nc.scalar.activation(out=feat[:, 0:6],
                                     in_=pooledr_sb[:, ex * 6:(ex + 1) * 6],
                                     func=AF.Copy)
                pt_v = pt[:].rearrange("p (c s) -> p c s", c=6)
                nc.vector.tensor_reduce(out=feat[:, 6:12], in_=pt_v, axis=AX.X,
                                        op=OP.max)
                nc.vector.tensor_scalar_max(out=feat[:, 6:12],
                                            in0=feat[:, 6:12], scalar1=0.0)

                # text-sum [1, H] -> h-partitioned [128, 6] via 6 tiny
                # transposes, so the cls dot is one [128, 18] mult+reduce
                pscp = foldp.tile([1, H], f32)
                nc.scalar.activation(out=pscp[:], in_=ps[:], func=AF.Copy)
                psT = pacc2.tile([P, 6], f32)
                for c in range(6):
                    nc.tensor.transpose(out=psT[:, c:c + 1],
                                        in_=pscp[0:1, c * P:(c + 1) * P],
                                        identity=ones_f[0:1, 0:1])
                nc.scalar.activation(out=feat[:, 12:18], in_=psT[:],
                                     func=AF.Copy)
                cprod = foldp.tile([P, 18], f32)
                nc.vector.tensor_tensor(out=cprod[:], in0=feat[:],
                                        in1=cwc_sb[:, ex * 18:(ex + 1) * 18],
                                        op=OP.mult)
                cidx = NE + ex
                nc.vector.tensor_reduce(out=rhs34[:, cidx:cidx + 1],
                                        in_=cprod[:], axis=AX.X, op=OP.add)

            # ---- window trees (bf16; masked entries are exactly 0) ----
            ws = winpool.tile([P, OB_R * H // 2], bf16)
            nc.vector.tensor_tensor(out=ws[:], in0=wt[:, 0:4 * H],
                                    in1=wt[:, 4 * H:8 * H], op=OP.add)
            nc.vector.tensor_tensor(out=ws[:, 0:2 * H], in0=ws[:, 0:2 * H],
                                    in1=ws[:, 2 * H:4 * H], op=OP.add)
            nc.vector.tensor_tensor(out=ws[:, 0:H], in0=ws[:, 0:H],
                                    in1=ws[:, H:2 * H], op=OP.add)
            nc.vector.tensor_tensor(out=wt[:, 0:4 * H], in0=wt[:, 0:4 * H],
                                    in1=wt[:, 4 * H:8 * H], op=OP.max)
            nc.vector.tensor_tensor(out=wt[:, 0:2 * H], in0=wt[:, 0:2 * H],
                                    in1=wt[:, 2 * H:4 * H], op=OP.max)
            nc.vector.tensor_tensor(out=wt[:, 0:H], in0=wt[:, 0:H],
                                    in1=wt[:, H:2 * H], op=OP.max)

        # transpose max/sum/center to h-partition layout
        gfeat = winpool.tile([P, 3 * 6 * NE], f32)   # [cT|maxT|sumT]
        ptM = pbig.tile([P, H], bf16, tag="ptw")
        for c in range(6):
            nc.tensor.transpose(out=ptM[:, c * P:(c + 1) * P],
                                in_=wt[:, c * P:(c + 1) * P],
                                identity=ident_bf[:])
        ptM_v = bass.AP(ptM[:].tensor, ptM[:].offset,
                        [ptM[:].ap[0], [P, 6], [1, NE], [NE, OB]])
        nc.vector.tensor_reduce(out=gfeat[:, 6 * NE:12 * NE], in_=ptM_v,
                                axis=AX.X, op=OP.max)
        nc.vector.tensor_scalar_max(out=gfeat[:, 6 * NE:12 * NE],
                                    in0=gfeat[:, 6 * NE:12 * NE], scalar1=0.0)
        ptS = pbig.tile([P, H], bf16, tag="ptw")
        for c in range(6):
            nc.tensor.transpose(out=ptS[:, c * P:(c + 1) * P],
                                in_=ws[:, c * P:(c + 1) * P],
                                identity=ident_bf[:])
        ptS_v = bass.AP(ptS[:].tensor, ptS[:].offset,
                        [ptS[:].ap[0], [P, 6], [1, NE], [NE, OB]])
        nc.vector.tensor_reduce(out=gfeat[:, 12 * NE:18 * NE], in_=ptS_v,
                                axis=AX.X, op=OP.add)
        # avg = sum / cnt  (per (ex,g) along free)
        icnt_b = bass.AP(invcnt_sb.tensor, invcnt_sb.offset,
                         [invcnt_sb.ap[0], [0, 6], [1, NE]])
        gf_s = bass.AP(gfeat[:].tensor, gfeat[:].offset + 12 * NE,
                       [gfeat[:].ap[0], [NE, 6], [1, NE]])
        nc.vector.tensor_tensor(out=gf_s, in0=gf_s, in1=icnt_b, op=OP.mult)
        ptC = pbigc.tile([P, 6 * NE], bf16)
        for c in range(6):
            nc.tensor.transpose(out=ptC[:, c * NE:(c + 1) * NE],
                                in_=ct[:, c * P:(c + 1) * P],
                                identity=ident_bf[0:NE, 0:NE])
        nc.scalar.activation(out=gfeat[:, 0:6 * NE], in_=ptC[:], func=AF.Copy)

        # combined gap dot: gfeat[p, (part, c, exg)] * W[part*H + c*128 + p]
        gw_b = bass.AP(gwt_sb.tensor, gwt_sb.offset,
                       [gwt_sb.ap[0], [6, 3], [1, 6], [0, NE]])
        gf_v = bass.AP(gfeat[:].tensor, gfeat[:].offset,
                       [gfeat[:].ap[0], [6 * NE, 3], [NE, 6], [1, NE]])
        nc.vector.tensor_tensor(out=gf_v, in0=gf_v, in1=gw_b, op=OP.mult)
        gf_r = bass.AP(gfeat[:].tensor, gfeat[:].offset,
                       [gfeat[:].ap[0], [1, NE], [NE, 18]])
        nc.vector.tensor_reduce(out=rhs34[:, 0:NE], in_=gf_r, axis=AX.X,
                                op=OP.add)

        # ---- final ones-matmul + single packed store (host unpacks) ----
        # pscore reuses a pacc buffer (same tag as ps; ex0's is long consumed)
        psc = pacc.tile([1, H], f32, tag="ps")
        pscore = psc[0:1, 0:NE + EX]
        nc.tensor.matmul(out=pscore, lhsT=ones_f[:], rhs=rhs34[:],
                         start=True, stop=True)
        sg = smalls.tile([1, NE + EX], f32)
        nc.scalar.activation(out=sg[:], in_=pscore, func=AF.Copy)
        nc.sync.dma_start(out=out_d, in_=sg[0:1, :])

    nc.compile()
    _BUILT = nc
    return nc


def _prep_core(seq_c, pooled_c, bm_c, gids_c, gW, cW):
    """Host-side per-core input prep. seq_c [EX,S,H] f32 (view), bm_c [EX,S]
    bool, gids_c [EX,G] int, gW [3H] f32, cW [3H] f32."""
    import ml_dtypes
    f32 = np.float32
    bf16 = np.dtype(ml_dtypes.bfloat16)
    COLS, NAUX = _aux_cols()

    x = np.ascontiguousarray(seq_c.reshape(EX * S, H)).astype(bf16)

    aux = np.zeros((P, NAUX), np.int32)

    def put(name, arr):
        lo, hi = COLS[name]
        a = np.ascontiguousarray(arr)
        v = a.view(np.int32)
        aux[:v.shape[0], lo:hi] = v

    # stream mask columns: col (ex*NCH + T)*GRP + j = mask of token
    # T*512 + p*4 + j of example ex
    tmask = np.ascontiguousarray(
        bm_c.astype(f32).reshape(EX, NCH, P, GRP)
        .transpose(2, 0, 1, 3).reshape(P, NMC))
    put("tmaskf", tmask)
    put("tmaskb", tmask.astype(bf16))

    # window partitions: p = ob*32 + ex*16 + g; each reads OB_R=8 whole
    # rows starting at row r2 + ob*8 of a 32-row padded window
    obv = np.repeat(np.arange(OB), NE)            # [P]
    exv = np.tile(np.repeat(np.arange(EX), G), OB)
    gv = np.tile(np.arange(G), EX * OB)
    gid_p = gids_c[exv, gv]                       # [P]
    r2 = np.clip(gid_p - WIN, 0, S - OB * OB_R)   # [P] padded-window start
    put("winidx", (exv * S + r2 + obv * OB_R).astype(np.int32).reshape(P, 1))
    rows_w = (r2 + obv * OB_R)[:, None] + np.arange(OB_R)[None, :]  # [P, 8]
    inwin = (rows_w >= gid_p[:, None] - WIN) & (rows_w <= gid_p[:, None] + WIN)
    put("wmaskf", (bm_c[exv[:, None], rows_w] & inwin).astype(f32))

    # per-(ex,g) valid counts over the full 32 rows
    exg_e = np.repeat(np.arange(EX), G)
    exg_g = np.tile(np.arange(G), EX)
    gid_f = gids_c[exg_e, exg_g]
    r2f = np.clip(gid_f - WIN, 0, S - OB * OB_R)
    rows_f = r2f[:, None] + np.arange(OB * OB_R)[None, :]    # [NE, 32]
    inwin_f = (rows_f >= gid_f[:, None] - WIN) & (rows_f <= gid_f[:, None] + WIN)
    cnt = (bm_c[exg_e[:, None], rows_f] & inwin_f).sum(1).astype(f32)  # [NE]
    put("invcnt", np.broadcast_to(1.0 / cnt, (P, NE)).astype(f32))
    put("ctridx", (exg_e * S + gid_f).astype(np.int32).reshape(NE, 1))

    # gwt[p, part*6 + c] = W[part*H + c*128 + p]
    put("gwt", np.ascontiguousarray(
        gW.reshape(3, 6, P).transpose(2, 0, 1).reshape(P, 18), dtype=f32))

    tn = bm_c.sum(1).astype(f32)                  # [EX]
    cw12 = cW[:2 * H].reshape(2, 6, P)            # [part, c, p]
    cw3 = cW[2 * H:].reshape(6, P)                # [c, p]
    cwc = np.empty((P, EX * 18), f32)
    pooledr = np.empty((P, EX * 6), f32)
    for ex in range(EX):
        cwc[:, ex * 18:ex * 18 + 6] = cw12[0].T
        cwc[:, ex * 18 + 6:ex * 18 + 12] = cw12[1].T
        cwc[:, ex * 18 + 12:ex * 18 + 18] = cw3.T / tn[ex]
        pooledr[:, ex * 6:(ex + 1) * 6] = pooled_c[ex].reshape(6, P).T
    put("cwc", cwc)
    put("pooledr", pooledr)

    return {"x": x, "aux": aux}


def _make_in_maps(sequence_output, pooled_output, token_type_ids, word_mask,
                  gap_ids, gap_W, cls_W):
    seq = np.asarray(sequence_output, dtype=np.float32)
    pooled = np.asarray(pooled_output, dtype=np.float32)
    tti = np.asarray(token_type_ids)
    wmk = np.asarray(word_mask)
    gids = np.asarray(gap_ids).astype(np.int64)
    gW = np.asarray(gap_W, dtype=np.float32)
    cW = np.asarray(cls_W, dtype=np.float32)
    base_mask = (tti == 0) & (wmk != 0)

    in_maps = []
    for c in range(NCORES):
        lo = c * EX
        in_maps.append(_prep_core(seq[lo:lo + EX], pooled[lo:lo + EX],
                                  base_mask[lo:lo + EX], gids[lo:lo + EX],
                                  gW, cW))
    return in_maps, None


def _run(in_maps, nf=None, trace=False, trace_cores=None):
    from concourse import bass_utils
    del nf
    nc = _build()
    return bass_utils.run_bass_kernel_spmd(
        nc, in_maps, core_ids=list(range(NCORES)), trace=trace,
        trace_cores=trace_cores)


def kernel(sequence_output, pooled_output, token_type_ids, word_mask,
           gap_ids, gap_W, gap_b, cls_W, cls_b):
    in_maps, _ = _make_in_maps(sequence_output, pooled_output,
                               token_type_ids, word_mask, gap_ids,
                               gap_W, cls_W)
    res = _run(in_maps)
    out = np.empty((B, NOUT), np.float32)
    for c in range(NCORES):
        sg = res.results[c]["out"].reshape(NE + EX)
        for ex in range(EX):
            out[c * EX + ex, 0] = sg[NE + ex]
            out[c * EX + ex, 1:] = sg[ex * G:(ex + 1) * G]
    out[:, 0] += float(np.asarray(cls_b))
    out[:, 1:] += float(np.asarray(gap_b))
    return out.astype(np.float32)


# revision 32
# speedup vs baseline: 1.0283x; 1.0283x over previous
"""Trainium2 Bass kernel for nn_BertGTHead_37177236914708 (BertGT pooling head).

Full-input contract: kernel(**inputs) takes the complete (unsharded) numpy
inputs and returns the full [B, 1+G] float32 output.

Strategy (data-parallel over batch, 2 examples per NeuronCore, 8 cores):
  - x is shipped to DRAM as bf16 (tolerance is 2e-2; bf16 keeps us ~1e-3),
    halving HBM traffic and doubling VectorE throughput. All streaming is
    plain HWDGE (sync) DMA - no SWDGE emission on the critical path.
  - text pooling streams [128, 4-row] chunks; masked sums accumulate on the
    PE with the bf16 0/1 mask column as the stationary operand (start/stop
    PSUM groups); the masked values for the running max are produced by
    per-partition-scalar multiplies split between ScalarE (3 of 4 slots)
    and VectorE tensor_scalar at 4x (1 slot); running elementwise max on
    VectorE in bf16 (exact); partition-axis max finalized via PE transposes
    + one free-axis reduce.
  - the text-sum [1, H] PSUM row is transposed (6 tiny PE transposes) into
    h-partitioned layout so the cls dot is a single [128, 18] mult+reduce.
  - window pooling: 32-row padded windows gathered by ONE indirect DMA as
    [(ob,ex,g) partitions x 8 whole rows]; mask applied via VectorE
    tensor_scalar (exact 0/1); within-partition sum/max TT trees in bf16;
    cross-block reduction via PE transposes to h-partitioned layout; center
    rows by a second tiny indirect gather, also transposed.
  - final scores: one combined per-partition dot (center|max|avg vs relaid
    weights) reduced on VectorE, then a single ones-matmul sums the 128
    h-partials for all 34 outputs at once; host unpacks the packed row.
  - all small aux inputs (indices, masks, weights) ride ONE packed DMA.

Everything index/mask-shaped is precomputed on the host; all O(B*S*H) math
runs on the NeuronCores.
"""

import numpy as np
from contextlib import ExitStack

# ---- problem constants (hardcoded; harness runs kernel.py standalone) ----
B, S, H, G = 16, 4096, 768, 16
WIN = 15
WLEN = 2 * WIN + 1           # 31
NCORES = 8
EX = B // NCORES             # 2 examples per core
P = 128
GRP = 4                      # token rows per partition per stream chunk
NCH = S // (P * GRP)         # 8 stream chunks per example
OB = 4                       # 8-row blocks per (32-row padded) window
OB_R = 8                     # rows per block
NE = EX * G                  # 32 (ex, gap) pairs per core
NOUT = 1 + G                 # 17 scores per example
NMC = EX * NCH * GRP         # 64 stream mask columns

_BUILT = None
DEBUG = False


def _aux_cols():
    """Column offsets of the packed [P, K] i32 aux tensor (widths in i32)."""
    c = {}
    o = 0
    for name, w in [("winidx", 1), ("ctridx", 1), ("invcnt", NE),
                    ("gwt", 18), ("pooledr", EX * 6), ("cwc", EX * 18),
                    ("tmaskf", NMC), ("tmaskb", NMC // 2),
                    ("wmaskf", OB_R)]:
        c[name] = (o, o + w)
        o += w
    return c, o


def _build():
    """Build + compile the per-core Bass program (cached)."""
    global _BUILT
    if _BUILT is not None:
        return _BUILT

    import concourse.bacc as bacc
    import concourse.bass as bass
    import concourse.tile as tile
    from concourse import mybir
    from concourse.masks import make_identity

    f32 = mybir.dt.float32
    bf16 = mybir.dt.bfloat16
    i32 = mybir.dt.int32
    AF = mybir.ActivationFunctionType
    OP = mybir.AluOpType
    AX = mybir.AxisListType
    IOA = bass.IndirectOffsetOnAxis

    COLS, NAUX = _aux_cols()

    nc = bacc.Bacc("TRN2", target_bir_lowering=False, debug=False,
                   num_devices=NCORES)

    x_d = nc.dram_tensor("x", [EX * S, H], bf16, kind="ExternalInput").ap()
    aux_d = nc.dram_tensor("aux", [P, NAUX], i32, kind="ExternalInput").ap()
    out_d = nc.dram_tensor("out", [NE + EX], f32, kind="ExternalOutput").ap()

    with tile.TileContext(nc) as tc, ExitStack() as ctx:
        singles = ctx.enter_context(tc.tile_pool(name="singles", bufs=1))
        xpool = ctx.enter_context(tc.tile_pool(name="xin", bufs=6))
        xmpool = ctx.enter_context(tc.tile_pool(name="xm", bufs=4))
        accpool = ctx.enter_context(tc.tile_pool(name="acc", bufs=2))
        winpool = ctx.enter_context(tc.tile_pool(name="win", bufs=1))
        smalls = ctx.enter_context(tc.tile_pool(name="smalls", bufs=4))
        foldp = ctx.enter_context(tc.tile_pool(name="fold", bufs=4))
        pacc = ctx.enter_context(tc.tile_pool(name="pacc", bufs=2, space="PSUM"))
        pacc2 = ctx.enter_context(tc.tile_pool(name="pacc2", bufs=1, space="PSUM"))
        pbig = ctx.enter_context(tc.tile_pool(name="pbig", bufs=2, space="PSUM"))
        pbigc = ctx.enter_context(tc.tile_pool(name="pbigc", bufs=1, space="PSUM"))

        # ---- ONE packed aux dma (first: it gates the window gathers) ----
        aux_sb = singles.tile([P, NAUX], i32)
        nc.sync.dma_start(out=aux_sb[:], in_=aux_d)

        def aux(name, dtype=None, rows=None):
            lo, hi = COLS[name]
            ap = aux_sb[0:rows, lo:hi] if rows else aux_sb[:, lo:hi]
            return ap.bitcast(dtype) if dtype else ap

        winidx_sb = aux("winidx")
        ctridx_sb = aux("ctridx", rows=NE)
        invcnt_sb = aux("invcnt", f32)
        gwt_sb = aux("gwt", f32)
        pooledr_sb = aux("pooledr", f32)
        cwc_sb = aux("cwc", f32)
        tmaskf_sb = aux("tmaskf", f32)       # [P, NMC] f32 (ACT scale)
        tmaskb_sb = aux("tmaskb", bf16)      # [P, NMC] bf16 (PE lhsT / DVE TS)
        wmaskf_sb = aux("wmaskf", f32)       # [P, OB_R] f32

        # rhs of the final ones-matmul: cols 0..NE-1 = per-(ex,g) gap-score
        # partials (per h'-partition), col NE+ex = cls partials
        rhs34 = smalls.tile([P, NE + EX], f32)

        xrow = bass.AP(x_d.tensor, 0, [[H, EX * S], [1, H]])
        x3 = bass.AP(x_d.tensor, 0, [[GRP * H, EX * S // GRP], [1, GRP * H]])

        # ---- window gathers FIRST on the Pool queue, so wt lands while
        # VectorE is still idle and the window trees fill the warm-up gap
        wt = winpool.tile([P, OB_R * H], bf16)
        nc.gpsimd.indirect_dma_start(
            out=wt[:], out_offset=None, in_=xrow,
            in_offset=IOA(ap=winidx_sb, axis=0))
        ct = winpool.tile([NE, H], bf16)
        nc.gpsimd.indirect_dma_start(
            out=ct[:], out_offset=None, in_=xrow,
            in_offset=IOA(ap=ctridx_sb, axis=0))

        ident_bf = singles.tile([P, P], bf16)
        make_identity(nc, ident_bf[:])
        ones_f = singles.tile([P, 1], f32)
        nc.vector.memset(ones_f[:], 1.0)

        with nc.allow_low_precision(reason="bf16 pooling; tol 2e-2"):
            # window mask-mul + trees on GpSimd: it is idle in the stream
            # phase while VectorE is the bottleneck engine
            for o in range(OB_R):
                nc.vector.tensor_scalar_mul(out=wt[:, o * H:(o + 1) * H],
                                            in0=wt[:, o * H:(o + 1) * H],
                                            scalar1=wmaskf_sb[:, o:o + 1])

            # ---- streaming text-pooling phase ----
            for ex in range(EX):
                acc = accpool.tile([P, GRP * H], bf16)
                ps = pacc.tile([1, H], f32, tag="ps")
                for T in range(NCH):
                    xt = xpool.tile([P, GRP * H], bf16, tag="xt")
                    row0 = (ex * S + T * P * GRP) // GRP
                    nc.sync.dma_start(out=xt[:], in_=x3[row0:row0 + P, :])
                    xm = acc if T == 0 else xmpool.tile([P, GRP * H], bf16,
                                                        tag="xm")
                    first = T == 0
                    last = T == NCH - 1
                    for j in range(GRP):
                        c = (ex * NCH + T) * GRP + j
                        mb = tmaskb_sb[:, c:c + 1]
                        nc.tensor.matmul(out=ps[0:1, 0:512], lhsT=mb,
                                         rhs=xt[:, j * H:j * H + 512],
                                         start=first and j == 0,
                                         stop=last and j == GRP - 1)
                        nc.tensor.matmul(out=ps[0:1, 512:H], lhsT=mb,
                                         rhs=xt[:, j * H + 512:(j + 1) * H],
                                         start=first and j == 0,
                                         stop=last and j == GRP - 1)
                        dst = xm[:, j * H:(j + 1) * H]
                        src = xt[:, j * H:(j + 1) * H]
                        if j == GRP - 1 or (j == 2 and T < 2):
                            nc.vector.tensor_scalar_mul(
                                out=dst, in0=src,
                                scalar1=tmaskf_sb[:, c:c + 1])
                        else:
                            nc.scalar.activation(out=dst, in_=src,
                                                 func=AF.Copy,
                                                 scale=tmaskf_sb[:, c:c + 1])
                    if T > 0:
                        nc.vector.tensor_tensor(out=acc[:], in0=acc[:],
                                                in1=xm[:], op=OP.max)

                # fold the GRP slots: maxf[p, h] = max_j acc[p, j*H + h]
                nc.vector.tensor_tensor(out=acc[:, 0:2 * H],
                                        in0=acc[:, 0:2 * H],
                                        in1=acc[:, 2 * H:4 * H], op=OP.max)
                maxf = foldp.tile([P, H], bf16)
                nc.vector.tensor_tensor(out=maxf[:], in0=acc[:, 0:H],
                                        in1=acc[:, H:2 * H], op=OP.max)

                # transpose to h-partition layout; reduce the 128 token rows
                pt = pbig.tile([P, H], bf16, tag="ptw")
                for c in range(6):
                    nc.tensor.transpose(out=pt[:, c * P:(c + 1) * P],
                                        in_=maxf[:, c * P:(c + 1) * P],
                                        identity=ident_bf[:])
                feat = foldp.tile([P, 18], f32)
                nc.scalar.activation(out=feat[:, 0:6],
                                     in_=pooledr_sb[:, ex * 6:(ex + 1) * 6],
                                     func=AF.Copy)
                pt_v = pt[:].rearrange("p (c s) -> p c s", c=6)
                nc.vector.tensor_reduce(out=feat[:, 6:12], in_=pt_v, axis=AX.X,
                                        op=OP.max)
                nc.vector.tensor_scalar_max(out=feat[:, 6:12],
                                            in0=feat[:, 6:12], scalar1=0.0)

                # text-sum [1, H] -> h-partitioned [128, 6] via 6 tiny
                # transposes, so the cls dot is one [128, 18] mult+reduce
                pscp = foldp.tile([1, H], f32)
                nc.scalar.activation(out=pscp[:], in_=ps[:], func=AF.Copy)
                psT = pacc2.tile([P, 6], f32)
                for c in range(6):
                    nc.tensor.transpose(out=psT[:, c:c + 1],
                                        in_=pscp[0:1, c * P:(c + 1) * P],
                                        identity=ones_f[0:1, 0:1])
                nc.scalar.activation(out=feat[:, 12:18], in_=psT[:],
                                     func=AF.Copy)
                cprod = foldp.tile([P, 18], f32)
                nc.vector.tensor_tensor(out=cprod[:], in0=feat[:],
                                        in1=cwc_sb[:, ex * 18:(ex + 1) * 18],
                                        op=OP.mult)
                cidx = NE + ex
                nc.vector.tensor_reduce(out=rhs34[:, cidx:cidx + 1],
                                        in_=cprod[:], axis=AX.X, op=OP.add)

            # ---- window trees (bf16; masked entries are exactly 0) ----
            ws = winpool.tile([P, OB_R * H // 2], bf16)
            nc.vector.tensor_tensor(out=ws[:], in0=wt[:, 0:4 * H],
                                    in1=wt[:, 4 * H:8 * H], op=OP.add)
            nc.vector.tensor_tensor(out=ws[:, 0:2 * H], in0=ws[:, 0:2 * H],
                                    in1=ws[:, 2 * H:4 * H], op=OP.add)
            nc.vector.tensor_tensor(out=ws[:, 0:H], in0=ws[:, 0:H],
                                    in1=ws[:, H:2 * H], op=OP.add)
            nc.vector.tensor_tensor(out=wt[:, 0:4 * H], in0=wt[:, 0:4 * H],
                                    in1=wt[:, 4 * H:8 * H], op=OP.max)
            nc.vector.tensor_tensor(out=wt[:, 0:2 * H], in0=wt[:, 0:2 * H],
                                    in1=wt[:, 2 * H:4 * H], op=OP.max)
            nc.vector.tensor_tensor(out=wt[:, 0:H], in0=wt[:, 0:H],
                                    in1=wt[:, H:2 * H], op=OP.max)

        # transpose max/sum/center to h-partition layout
        gfeat = winpool.tile([P, 3 * 6 * NE], f32)   # [cT|maxT|sumT]
        ptM = pbig.tile([P, H], bf16, tag="ptw")
        for c in range(6):
            nc.tensor.transpose(out=ptM[:, c * P:(c + 1) * P],
                                in_=wt[:, c * P:(c + 1) * P],
                                identity=ident_bf[:])
        ptM_v = bass.AP(ptM[:].tensor, ptM[:].offset,
                        [ptM[:].ap[0], [P, 6], [1, NE], [NE, OB]])
        nc.vector.tensor_reduce(out=gfeat[:, 6 * NE:12 * NE], in_=ptM_v,
                                axis=AX.X, op=OP.max)
        nc.vector.tensor_scalar_max(out=gfeat[:, 6 * NE:12 * NE],
                                    in0=gfeat[:, 6 * NE:12 * NE], scalar1=0.0)
        ptS = pbig.tile([P, H], bf16, tag="ptw")
        for c in range(6):
            nc.tensor.transpose(out=ptS[:, c * P:(c + 1) * P],
                                in_=ws[:, c * P:(c + 1) * P],
                                identity=ident_bf[:])
        ptS_v = bass.AP(ptS[:].tensor, ptS[:].offset,
                        [ptS[:].ap[0], [P, 6], [1, NE], [NE, OB]])
        nc.vector.tensor_reduce(out=gfeat[:, 12 * NE:18 * NE], in_=ptS_v,
                                axis=AX.X, op=OP.add)
        # avg = sum / cnt  (per (ex,g) along free)
        icnt_b = bass.AP(invcnt_sb.tensor, invcnt_sb.offset,
                         [invcnt_sb.ap[0], [0, 6], [1, NE]])
        gf_s = bass.AP(gfeat[:].tensor, gfeat[:].offset + 12 * NE,
                       [gfeat[:].ap[0], [NE, 6], [1, NE]])
        nc.vector.tensor_tensor(out=gf_s, in0=gf_s, in1=icnt_b, op=OP.mult)
        ptC = pbigc.tile([P, 6 * NE], bf16)
        for c in range(6):
            nc.tensor.transpose(out=ptC[:, c * NE:(c + 1) * NE],
                                in_=ct[:, c * P:(c + 1) * P],
                                identity=ident_bf[0:NE, 0:NE])
        nc.scalar.activation(out=gfeat[:, 0:6 * NE], in_=ptC[:], func=AF.Copy)

        # combined gap dot: gfeat[p, (part, c, exg)] * W[part*H + c*128 + p]
        gw_b = bass.AP(gwt_sb.tensor, gwt_sb.offset,
                       [gwt_sb.ap[0], [6, 3], [1, 6], [0, NE]])
        gf_v = bass.AP(gfeat[:].tensor, gfeat[:].offset,
                       [gfeat[:].ap[0], [6 * NE, 3], [NE, 6], [1, NE]])
        nc.vector.tensor_tensor(out=gf_v, in0=gf_v, in1=gw_b, op=OP.mult)
        gf_r = bass.AP(gfeat[:].tensor, gfeat[:].offset,
                       [gfeat[:].ap[0], [1, NE], [NE, 18]])
        nc.vector.tensor_reduce(out=rhs34[:, 0:NE], in_=gf_r, axis=AX.X,
                                op=OP.add)

        # ---- final ones-matmul + single packed store (host unpacks) ----
        # pscore reuses a pacc buffer (same tag as ps; ex0's is long consumed)
        psc = pacc.tile([1, H], f32, tag="ps")
        pscore = psc[0:1, 0:NE + EX]
        nc.tensor.matmul(out=pscore, lhsT=ones_f[:], rhs=rhs34[:],
                         start=True, stop=True)
        sg = smalls.tile([1, NE + EX], f32)
        nc.scalar.activation(out=sg[:], in_=pscore, func=AF.Copy)
        nc.sync.dma_start(out=out_d, in_=sg[0:1, :])

    nc.compile()
    _BUILT = nc
    return nc


def _prep_core(seq_c, pooled_c, bm_c, gids_c, gW, cW):
    """Host-side per-core input prep. seq_c [EX,S,H] f32 (view), bm_c [EX,S]
    bool, gids_c [EX,G] int, gW [3H] f32, cW [3H] f32."""
    import ml_dtypes
    f32 = np.float32
    bf16 = np.dtype(ml_dtypes.bfloat16)
    COLS, NAUX = _aux_cols()

    x = np.ascontiguousarray(seq_c.reshape(EX * S, H)).astype(bf16)

    aux = np.zeros((P, NAUX), np.int32)

    def put(name, arr):
        lo, hi = COLS[name]
        a = np.ascontiguousarray(arr)
        v = a.view(np.int32)
        aux[:v.shape[0], lo:hi] = v

    # stream mask columns: col (ex*NCH + T)*GRP + j = mask of token
    # T*512 + p*4 + j of example ex
    tmask = np.ascontiguousarray(
        bm_c.astype(f32).reshape(EX, NCH, P, GRP)
        .transpose(2, 0, 1, 3).reshape(P, NMC))
    put("tmaskf", tmask)
    put("tmaskb", tmask.astype(bf16))

    # window partitions: p = ob*32 + ex*16 + g; each reads OB_R=8 whole
    # rows starting at row r2 + ob*8 of a 32-row padded window
    obv = np.repeat(np.arange(OB), NE)            # [P]
    exv = np.tile(np.repeat(np.arange(EX), G), OB)
    gv = np.tile(np.arange(G), EX * OB)
    gid_p = gids_c[exv, gv]                       # [P]
    r2 = np.clip(gid_p - WIN, 0, S - OB * OB_R)   # [P] padded-window start
    put("winidx", (exv * S + r2 + obv * OB_R).astype(np.int32).reshape(P, 1))
    rows_w = (r2 + obv * OB_R)[:, None] + np.arange(OB_R)[None, :]  # [P, 8]
    inwin = (rows_w >= gid_p[:, None] - WIN) & (rows_w <= gid_p[:, None] + WIN)
    put("wmaskf", (bm_c[exv[:, None], rows_w] & inwin).astype(f32))

    # per-(ex,g) valid counts over the full 32 rows
    exg_e = np.repeat(np.arange(EX), G)
    exg_g = np.tile(np.arange(G), EX)
    gid_f = gids_c[exg_e, exg_g]
    r2f = np.clip(gid_f - WIN, 0, S - OB * OB_R)
    rows_f = r2f[:, None] + np.arange(OB * OB_R)[None, :]    # [NE, 32]
    inwin_f = (rows_f >= gid_f[:, None] - WIN) & (rows_f <= gid_f[:, None] + WIN)
    cnt = (bm_c[exg_e[:, None], rows_f] & inwin_f).sum(1).astype(f32)  # [NE]
    put("invcnt", np.broadcast_to(1.0 / cnt, (P, NE)).astype(f32))
    put("ctridx", (exg_e * S + gid_f).astype(np.int32).reshape(NE, 1))

    # gwt[p, part*6 + c] = W[part*H + c*128 + p]
    put("gwt", np.ascontiguousarray(
        gW.reshape(3, 6, P).transpose(2, 0, 1).reshape(P, 18), dtype=f32))

    tn = bm_c.sum(1).astype(f32)                  # [EX]
    cw12 = cW[:2 * H].reshape(2, 6, P)            # [part, c, p]
    cw3 = cW[2 * H:].reshape(6, P)                # [c, p]
    cwc = np.empty((P, EX * 18), f32)
    pooledr = np.empty((P, EX * 6), f32)
    for ex in range(EX):
        cwc[:, ex * 18:ex * 18 + 6] = cw12[0].T
        cwc[:, ex * 18 + 6:ex * 18 + 12] = cw12[1].T
        cwc[:, ex * 18 + 12:ex * 18 + 18] = cw3.T / tn[ex]
        pooledr[:, ex * 6:(ex + 1) * 6] = pooled_c[ex].reshape(6, P).T
    put("cwc", cwc)
    put("pooledr", pooledr)

    return {"x": x, "aux": aux}


def _make_in_maps(sequence_output, pooled_output, token_type_ids, word_mask,
                  gap_ids, gap_W, cls_W):
    seq = np.asarray(sequence_output, dtype=np.float32)
    pooled = np.asarray(pooled_output, dtype=np.float32)
    tti = np.asarray(token_type_ids)
    wmk = np.asarray(word_mask)
    gids = np.asarray(gap_ids).astype(np.int64)
    gW = np.asarray(gap_W, dtype=np.float32)
    cW = np.asarray(cls_W, dtype=np.float32)
    base_mask = (tti == 0) & (wmk != 0)

    in_maps = []
    for c in range(NCORES):
        lo = c * EX
        in_maps.append(_prep_core(seq[lo:lo + EX], pooled[lo:lo + EX],
                                  base_mask[lo:lo + EX], gids[lo:lo + EX],
                                  gW, cW))
    return in_maps, None


def _run(in_maps, nf=None, trace=False, trace_cores=None):
    from concourse import bass_utils
    del nf
    nc = _build()
    return bass_utils.run_bass_kernel_spmd(
        nc, in_maps, core_ids=list(range(NCORES)), trace=trace,
        trace_cores=trace_cores)


def kernel(sequence_output, pooled_output, token_type_ids, word_mask,
           gap_ids, gap_W, gap_b, cls_W, cls_b):
    in_maps, _ = _make_in_maps(sequence_output, pooled_output,
                               token_type_ids, word_mask, gap_ids,
                               gap_W, cls_W)
    res = _run(in_maps)
    out = np.empty((B, NOUT), np.float32)
    for c in range(NCORES):
        sg = res.results[c]["out"].reshape(NE + EX)
        for ex in range(EX):
            out[c * EX + ex, 0] = sg[NE + ex]
            out[c * EX + ex, 1:] = sg[ex * G:(ex + 1) * G]
    out[:, 0] += float(np.asarray(cls_b))
    out[:, 1:] += float(np.asarray(gap_b))
    return out.astype(np.float32)
